# revision 25
# baseline (speedup 1.0000x reference)
"""Trainium2 Bass kernel for nn_C4TransformerVM (neural-ALU 4-byte one-hot adder).

The reference module performs 32-bit addition over one-hot byte encodings via
LUT matmuls + softmax(TEMP=100). With exact one-hot inputs the softmax output
is one-hot to within ~4e-44 (fp32 subnormals), so an exact
decode -> integer ripple-carry add -> one-hot encode pipeline matches the
reference to ~1e-43 relative error while touching each input/output element
exactly once (memory-bound regime).

Per core (data-parallel over batch, 4096 tokens each):
  decode: s = a+b (DVE wide add), then fused multiply-by-iota + reduce
          (tensor_tensor_reduce) -> per-token byte sums av+bv (exact fp32 ints)
  carry:  batched ripple-carry on [128, G] tiles
  encode: tensor_scalar is_equal(iota_row, r) on GPSIMD -> one-hot rows
"""

from contextlib import ExitStack

import numpy as np

import concourse.bacc as bacc
import concourse.tile as tile
from concourse import mybir
from concourse.bass_utils import run_bass_kernel_spmd

N_CORES = 8
N_TOK = 32768
P = 128
TOK_PER_CORE = N_TOK // N_CORES  # 4096


def build_nc(
    n_tok_core=TOK_PER_CORE,
    g_blocks=8,
    n_cores=N_CORES,
    n_reps=None,
    variant="full",
):
    """Build + compile the per-core Bass program (SPMD: same on all cores).

    n_reps: if set, wrap the whole body in a runtime For_i loop that re-executes
    it n_reps times (for on-device timing via wall-clock differencing).
    variant: "full" | "dmaonly" (loads+stores, no compute) | "noenc"
    (decode+carry but store s instead of encoding) | "acte" (encode on ACT).
    """
    nb = n_tok_core // P  # 128-token blocks per core
    g = min(g_blocks, nb)  # blocks per pipeline group
    assert nb % g == 0

    f32 = mybir.dt.float32
    op = mybir.AluOpType

    nc = bacc.Bacc(
        "TRN2",
        target_bir_lowering=False,
        debug=False,
        num_devices=n_cores,
    )
    a_d = nc.dram_tensor("a", [4, n_tok_core, 256], f32, kind="ExternalInput").ap()
    b_d = nc.dram_tensor("b", [4, n_tok_core, 256], f32, kind="ExternalInput").ap()
    iota_d = nc.dram_tensor("iota", [P, 256], f32, kind="ExternalInput").ap()
    o_d = nc.dram_tensor("o", [4, n_tok_core, 256], f32, kind="ExternalOutput").ap()

    with tile.TileContext(nc) as tc, ExitStack() as ctx:
        const = ctx.enter_context(tc.tile_pool(name="const", bufs=1))
        io = ctx.enter_context(tc.tile_pool(name="io", bufs=4))
        work = ctx.enter_context(tc.tile_pool(name="work", bufs=3))
        vals = ctx.enter_context(tc.tile_pool(name="vals", bufs=3))
        enc = ctx.enter_context(tc.tile_pool(name="enc", bufs=4))
        psum = ctx.enter_context(tc.tile_pool(name="psum", bufs=6, space="PSUM"))

        iota_t = const.tile([P, 256], f32)
        nc.sync.dma_start(iota_t[:], iota_d)

        rep_ctx = (
            tc.For_i(
                0,
                n_reps,
                1,
                staggered_reset=True,
                hint_engines=(
                    mybir.EngineType.DVE,
                    mybir.EngineType.Activation,
                    mybir.EngineType.SP,
                ),
            )
            if n_reps is not None
            else None
        )
        if rep_ctx is not None:
            ctx.enter_context(rep_ctx)

        for gi in range(nb // g):
            tok0 = gi * g * P
            ntok_g = g * P

            if variant.startswith("dma"):
                if variant == "dma2":
                    load_engs, store_eng = (nc.sync, nc.sync), nc.scalar
                elif variant == "dma3":
                    load_engs, store_eng = (nc.sync, nc.sync), nc.gpsimd
                elif variant == "dma4":
                    load_engs, store_eng = (nc.sync, nc.scalar), nc.gpsimd
                else:
                    load_engs, store_eng = (nc.sync, nc.sync), nc.sync
                for i in range(4):
                    a_v = a_d[i, tok0 : tok0 + ntok_g, :].rearrange(
                        "(p c) f -> p c f", p=P
                    )
                    b_v = b_d[i, tok0 : tok0 + ntok_g, :].rearrange(
                        "(p c) f -> p c f", p=P
                    )
                    a_t = io.tile([P, g, 256], f32, tag="a")
                    b_t = io.tile([P, g, 256], f32, tag="b")
                    load_engs[0].dma_start(a_t[:], a_v)
                    load_engs[1].dma_start(b_t[:], b_v)
                    o_v = o_d[i, tok0 : tok0 + ntok_g, :].rearrange(
                        "(p c) f -> p c f", p=P
                    )
                    store_eng.dma_start(o_v, a_t[:])
                continue

            # t0[:, i, c] = av + bv for byte i, block c (value 0..510)
            t0 = vals.tile([P, 4, g], f32, tag="t0")
            s_tiles = []
            for i in range(4):
                a_v = a_d[i, tok0 : tok0 + ntok_g, :].rearrange(
                    "(p c) f -> p c f", p=P
                )
                b_v = b_d[i, tok0 : tok0 + ntok_g, :].rearrange(
                    "(p c) f -> p c f", p=P
                )
                a_t = io.tile([P, g, 256], f32, tag="a")
                b_t = io.tile([P, g, 256], f32, tag="b")
                nc.sync.dma_start(a_t[:], a_v)
                nc.sync.dma_start(b_t[:], b_v)
                s_t = work.tile([P, g, 256], f32, tag="s")
                add_eng = nc.gpsimd if variant == "v3" else nc.vector
                add_eng.tensor_add(s_t[:], a_t[:], b_t[:])
                trash = work.tile([P, 256], f32, tag="trash")
                for c in range(g):
                    nc.vector.scalar_tensor_tensor(
                        out=trash[:],
                        in0=s_t[:, c, :],
                        scalar=1.0,
                        in1=iota_t[:],
                        op0=op.mult,
                        op1=op.mult,
                        accum_out=t0[:, i, c : c + 1],
                    )
                if variant == "noenc":
                    o_v = o_d[i, tok0 : tok0 + ntok_g, :].rearrange(
                        "(p c) f -> p c f", p=P
                    )
                    nc.sync.dma_start(o_v, s_t[:])

            if variant == "noenc":
                continue

            # ripple carry: t_i = t0_i + c_i ; c_{i+1} = t_i > 255 ; r_i = t_i - 256*c_{i+1}
            r = vals.tile([P, 4, g], f32, tag="r")
            c_t = vals.tile([P, 4, g], f32, tag="c")
            prev = None
            for i in range(4):
                if prev is None:
                    t_i = t0[:, i, :]
                else:
                    t_tile = vals.tile([P, g], f32, tag="t")
                    nc.vector.tensor_add(t_tile[:], t0[:, i, :], prev)
                    t_i = t_tile[:]
                nc.vector.tensor_scalar(
                    out=c_t[:, i, :], in0=t_i, scalar1=255.5, scalar2=None, op0=op.is_gt
                )
                if variant not in ("full",):
                    # rn = 256*c - t = -r  (bias for ACT-side |iota - r|)
                    nc.vector.scalar_tensor_tensor(
                        out=r[:, i, :],
                        in0=c_t[:, i, :],
                        scalar=256.0,
                        in1=t_i,
                        op0=op.mult,
                        op1=op.subtract,
                    )
                else:
                    nc.vector.scalar_tensor_tensor(
                        out=r[:, i, :],
                        in0=c_t[:, i, :],
                        scalar=-256.0,
                        in1=t_i,
                        op0=op.mult,
                        op1=op.add,
                    )
                prev = c_t[:, i, :]

            for i in range(4):
                o_t = enc.tile([P, g, 256], f32, tag="o")
                if variant not in ("full",):
                    for c in range(g):
                        d_t = psum.tile([P, 256], f32, tag="d")
                        nc.scalar.activation(
                            out=d_t[:],
                            in_=iota_t[:],
                            func=mybir.ActivationFunctionType.Abs,
                            bias=r[:, i, c : c + 1],
                            scale=1.0,
                        )
                        if (
                            variant == "v2"
                            or (variant in ("v4", "v6", "v7") and c % 4 == 0)
                            or (variant == "v6b" and c % 2 == 0)
                        ):
                            nc.vector.tensor_scalar(
                                out=o_t[:, c, :],
                                in0=d_t[:],
                                scalar1=0.5,
                                scalar2=None,
                                op0=op.is_lt,
                            )
                        else:
                            nc.scalar.activation(
                                out=o_t[:, c, :],
                                in_=d_t[:],
                                func=mybir.ActivationFunctionType.Relu,
                                bias=1.0,
                                scale=-1.0,
                            )
                else:
                    for c in range(g):
                        nc.gpsimd.tensor_scalar(
                            out=o_t[:, c, :],
                            in0=iota_t[:],
                            scalar1=r[:, i, c : c + 1],
                            scalar2=None,
                            op0=op.is_equal,
                        )
                o_v = o_d[i, tok0 : tok0 + ntok_g, :].rearrange(
                    "(p c) f -> p c f", p=P
                )
                store_eng = (
                    nc.gpsimd
                    if variant in ("v5", "v6", "v6b")
                    else (nc.scalar if variant == "v7" else nc.sync)
                )
                store_eng.dma_start(o_v, o_t[:])

    nc.compile()
    return nc


_NC_CACHE = {}


def _get_nc():
    key = (TOK_PER_CORE, N_CORES)
    if key not in _NC_CACHE:
        _NC_CACHE[key] = build_nc(variant="v6")
    return _NC_CACHE[key]


def make_in_maps(a, b, n_cores=N_CORES, n_tok_core=TOK_PER_CORE):
    iota = np.ascontiguousarray(
        np.broadcast_to(np.arange(256, dtype=np.float32), (P, 256))
    )
    in_maps = []
    for c in range(n_cores):
        sl = slice(c * n_tok_core, (c + 1) * n_tok_core)
        in_maps.append(
            {
                "a": np.ascontiguousarray(a[:, sl]),
                "b": np.ascontiguousarray(b[:, sl]),
                "iota": iota,
            }
        )
    return in_maps


def kernel(**inputs):
    a = np.asarray(inputs["a"], dtype=np.float32)
    b = np.asarray(inputs["b"], dtype=np.float32)
    nc = _get_nc()
    res = run_bass_kernel_spmd(nc, make_in_maps(a, b), core_ids=list(range(N_CORES)))
    return np.concatenate([res.results[c]["o"] for c in range(N_CORES)], axis=1)


# revision 27
# speedup vs baseline: 1.0019x; 1.0019x over previous
"""Trainium2 Bass kernel for nn_C4TransformerVM (neural-ALU 4-byte one-hot adder).

The reference module performs 32-bit addition over one-hot byte encodings via
LUT matmuls + softmax(TEMP=100). With exact one-hot inputs the softmax output
is one-hot to within ~4e-44 (fp32 subnormals), so an exact
decode -> integer ripple-carry add -> one-hot encode pipeline matches the
reference to ~1e-43 relative error while touching each input/output element
exactly once (memory-bound regime).

Per core (pure data parallel over the batch, 4096 tokens each; production
variant "v6", measured ~155us/core vs ~153us DMA floor for its 48MB of I/O):
  decode: s = a+b (DVE wide add), then fused multiply-by-iota + sum-reduce
          (scalar_tensor_tensor accum_out) -> per-token byte sums av+bv
          (exact fp32 integers)
  carry:  batched ripple-carry on [128, G] value tiles (DVE)
  encode: d = |iota - r| via ACT Abs with per-partition bias into PSUM, then
          one-hot = Relu(1-d) on ACT (3 of 4 chunks) or (d < 0.5) on DVE
          (every 4th chunk, balancing engine load)
  stores: issued from GPSIMD's SWDGE ring so they cannot head-of-line block
          loads on the sync-engine HWDGE ring
  layout: tokens -> (partition, chunk) with consecutive tokens per partition,
          so every 1MB DMA is 8KB-contiguous per partition
"""

from contextlib import ExitStack

import numpy as np

import concourse.bacc as bacc
import concourse.tile as tile
from concourse import mybir
from concourse.bass_utils import run_bass_kernel_spmd

N_CORES = 8
N_TOK = 32768
P = 128
TOK_PER_CORE = N_TOK // N_CORES  # 4096


def build_nc(
    n_tok_core=TOK_PER_CORE,
    g_blocks=8,
    n_cores=N_CORES,
    n_reps=None,
    variant="full",
):
    """Build + compile the per-core Bass program (SPMD: same on all cores).

    n_reps: if set, wrap the whole body in a runtime For_i loop that re-executes
    it n_reps times (for on-device timing via wall-clock differencing).
    variant: "full" | "dmaonly" (loads+stores, no compute) | "noenc"
    (decode+carry but store s instead of encoding) | "acte" (encode on ACT).
    """
    nb = n_tok_core // P  # 128-token blocks per core
    g = min(g_blocks, nb)  # blocks per pipeline group
    assert nb % g == 0

    f32 = mybir.dt.float32
    op = mybir.AluOpType

    nc = bacc.Bacc(
        "TRN2",
        target_bir_lowering=False,
        debug=False,
        num_devices=n_cores,
    )
    a_d = nc.dram_tensor("a", [4, n_tok_core, 256], f32, kind="ExternalInput").ap()
    b_d = nc.dram_tensor("b", [4, n_tok_core, 256], f32, kind="ExternalInput").ap()
    iota_d = nc.dram_tensor("iota", [P, 256], f32, kind="ExternalInput").ap()
    o_d = nc.dram_tensor("o", [4, n_tok_core, 256], f32, kind="ExternalOutput").ap()

    with tile.TileContext(nc) as tc, ExitStack() as ctx:
        const = ctx.enter_context(tc.tile_pool(name="const", bufs=1))
        io = ctx.enter_context(tc.tile_pool(name="io", bufs=4))
        work = ctx.enter_context(tc.tile_pool(name="work", bufs=3))
        vals = ctx.enter_context(tc.tile_pool(name="vals", bufs=3))
        enc = ctx.enter_context(tc.tile_pool(name="enc", bufs=4))
        psum = ctx.enter_context(tc.tile_pool(name="psum", bufs=6, space="PSUM"))

        iota_t = const.tile([P, 256], f32)
        nc.sync.dma_start(iota_t[:], iota_d)

        rep_ctx = (
            tc.For_i(
                0,
                n_reps,
                1,
                staggered_reset=True,
                hint_engines=(
                    mybir.EngineType.DVE,
                    mybir.EngineType.Activation,
                    mybir.EngineType.SP,
                ),
            )
            if n_reps is not None
            else None
        )
        if rep_ctx is not None:
            ctx.enter_context(rep_ctx)

        for gi in range(nb // g):
            tok0 = gi * g * P
            ntok_g = g * P

            if variant.startswith("dma"):
                if variant == "dma2":
                    load_engs, store_eng = (nc.sync, nc.sync), nc.scalar
                elif variant == "dma3":
                    load_engs, store_eng = (nc.sync, nc.sync), nc.gpsimd
                elif variant == "dma4":
                    load_engs, store_eng = (nc.sync, nc.scalar), nc.gpsimd
                else:
                    load_engs, store_eng = (nc.sync, nc.sync), nc.sync
                for i in range(4):
                    a_v = a_d[i, tok0 : tok0 + ntok_g, :].rearrange(
                        "(p c) f -> p c f", p=P
                    )
                    b_v = b_d[i, tok0 : tok0 + ntok_g, :].rearrange(
                        "(p c) f -> p c f", p=P
                    )
                    a_t = io.tile([P, g, 256], f32, tag="a")
                    b_t = io.tile([P, g, 256], f32, tag="b")
                    load_engs[0].dma_start(a_t[:], a_v)
                    load_engs[1].dma_start(b_t[:], b_v)
                    o_v = o_d[i, tok0 : tok0 + ntok_g, :].rearrange(
                        "(p c) f -> p c f", p=P
                    )
                    store_eng.dma_start(o_v, a_t[:])
                continue

            # t0[:, i, c] = av + bv for byte i, block c (value 0..510)
            t0 = vals.tile([P, 4, g], f32, tag="t0")
            for i in range(4):
                a_v = a_d[i, tok0 : tok0 + ntok_g, :].rearrange(
                    "(p c) f -> p c f", p=P
                )
                b_v = b_d[i, tok0 : tok0 + ntok_g, :].rearrange(
                    "(p c) f -> p c f", p=P
                )
                a_t = io.tile([P, g, 256], f32, tag="a")
                b_t = io.tile([P, g, 256], f32, tag="b")
                nc.sync.dma_start(a_t[:], a_v)
                nc.sync.dma_start(b_t[:], b_v)
                s_t = work.tile([P, g, 256], f32, tag="s")
                add_eng = nc.gpsimd if variant == "v3" else nc.vector
                add_eng.tensor_add(s_t[:], a_t[:], b_t[:])
                trash = work.tile([P, 256], f32, tag="trash")
                for c in range(g):
                    nc.vector.scalar_tensor_tensor(
                        out=trash[:],
                        in0=s_t[:, c, :],
                        scalar=1.0,
                        in1=iota_t[:],
                        op0=op.mult,
                        op1=op.mult,
                        accum_out=t0[:, i, c : c + 1],
                    )
                if variant == "noenc":
                    o_v = o_d[i, tok0 : tok0 + ntok_g, :].rearrange(
                        "(p c) f -> p c f", p=P
                    )
                    nc.sync.dma_start(o_v, s_t[:])

            if variant == "noenc":
                continue

            # ripple carry: t_i = t0_i + c_i ; c_{i+1} = t_i > 255 ; r_i = t_i - 256*c_{i+1}
            r = vals.tile([P, 4, g], f32, tag="r")
            c_t = vals.tile([P, 4, g], f32, tag="c")
            prev = None
            for i in range(4):
                if prev is None:
                    t_i = t0[:, i, :]
                else:
                    t_tile = vals.tile([P, g], f32, tag="t")
                    nc.vector.tensor_add(t_tile[:], t0[:, i, :], prev)
                    t_i = t_tile[:]
                nc.vector.tensor_scalar(
                    out=c_t[:, i, :], in0=t_i, scalar1=255.5, scalar2=None, op0=op.is_gt
                )
                if variant not in ("full",):
                    # rn = 256*c - t = -r  (bias for ACT-side |iota - r|)
                    nc.vector.scalar_tensor_tensor(
                        out=r[:, i, :],
                        in0=c_t[:, i, :],
                        scalar=256.0,
                        in1=t_i,
                        op0=op.mult,
                        op1=op.subtract,
                    )
                else:
                    nc.vector.scalar_tensor_tensor(
                        out=r[:, i, :],
                        in0=c_t[:, i, :],
                        scalar=-256.0,
                        in1=t_i,
                        op0=op.mult,
                        op1=op.add,
                    )
                prev = c_t[:, i, :]

            for i in range(4):
                o_t = enc.tile([P, g, 256], f32, tag="o")
                if variant not in ("full",):
                    for c in range(g):
                        d_t = psum.tile([P, 256], f32, tag="d")
                        nc.scalar.activation(
                            out=d_t[:],
                            in_=iota_t[:],
                            func=mybir.ActivationFunctionType.Abs,
                            bias=r[:, i, c : c + 1],
                            scale=1.0,
                        )
                        if (
                            variant == "v2"
                            or (variant in ("v4", "v6", "v7") and c % 4 == 0)
                            or (variant == "v6b" and c % 2 == 0)
                        ):
                            nc.vector.tensor_scalar(
                                out=o_t[:, c, :],
                                in0=d_t[:],
                                scalar1=0.5,
                                scalar2=None,
                                op0=op.is_lt,
                            )
                        else:
                            nc.scalar.activation(
                                out=o_t[:, c, :],
                                in_=d_t[:],
                                func=mybir.ActivationFunctionType.Relu,
                                bias=1.0,
                                scale=-1.0,
                            )
                else:
                    for c in range(g):
                        nc.gpsimd.tensor_scalar(
                            out=o_t[:, c, :],
                            in0=iota_t[:],
                            scalar1=r[:, i, c : c + 1],
                            scalar2=None,
                            op0=op.is_equal,
                        )
                o_v = o_d[i, tok0 : tok0 + ntok_g, :].rearrange(
                    "(p c) f -> p c f", p=P
                )
                store_eng = (
                    nc.gpsimd
                    if variant in ("v5", "v6", "v6b")
                    else (nc.scalar if variant == "v7" else nc.sync)
                )
                store_eng.dma_start(o_v, o_t[:])

    nc.compile()
    return nc


_NC_CACHE = {}


def _get_nc():
    key = (TOK_PER_CORE, N_CORES)
    if key not in _NC_CACHE:
        _NC_CACHE[key] = build_nc(variant="v6")
    return _NC_CACHE[key]


def make_in_maps(a, b, n_cores=N_CORES, n_tok_core=TOK_PER_CORE):
    iota = np.ascontiguousarray(
        np.broadcast_to(np.arange(256, dtype=np.float32), (P, 256))
    )
    in_maps = []
    for c in range(n_cores):
        sl = slice(c * n_tok_core, (c + 1) * n_tok_core)
        in_maps.append(
            {
                "a": np.ascontiguousarray(a[:, sl]),
                "b": np.ascontiguousarray(b[:, sl]),
                "iota": iota,
            }
        )
    return in_maps


def kernel(**inputs):
    a = np.asarray(inputs["a"], dtype=np.float32)
    b = np.asarray(inputs["b"], dtype=np.float32)
    nc = _get_nc()
    res = run_bass_kernel_spmd(nc, make_in_maps(a, b), core_ids=list(range(N_CORES)))
    return np.concatenate([res.results[c]["o"] for c in range(N_CORES)], axis=1)
